# revision 37
# baseline (speedup 1.0000x reference)
"""Trainium2 Bass kernel for nn_AblationRouter (moe_routing).

Computation (per batch row):
  h = EMA(x) with per-channel decay beta (constant 0.9 here)
  hid = relu([x, h] @ W1^T + b1);  route = hid @ W2^T + b2
  gates = softmax(route @ Wr^T + br)

Strategy: data-parallel over B=8 batch rows, one per NeuronCore.
W2 and Wr are both linear with no nonlinearity between them, so they are
folded on the host into Wc = Wr @ W2 [E, H] (and bc = br + Wr @ b2),
which removes the route matmul entirely: logits = hid @ Wc^T + bc.

The EMA recurrence is computed as matmuls against a precomputed decay
matrix over 512-token blocks with a 128-token lookback (beta^128 ~ 1e-6,
negligible vs matmul rounding -> no serial carry chain); the decay
matrix is banded so each 128-row s-chunk only streams the token range it
can reach.  All activations are channel-major ([channel, token]) so the
matmuls chain on the TensorEngine without transposes.

The abs split: relu(z) = z/2 + |z|/2, so
  logits = (1/2)(Wc W1)[x;h] + (1/2) Wc |pre + b1| + (bc + (1/2) Wc b1)
The first term is a rank-64 path (A' = Wc W1 / 2, 16 small matmuls per
token tile, f16 -- exact).  Only the |.| path carries fp8 quantization
noise, and |pre+n|-|pre| = sign(pre)*n has half the noise power of
relu's mask*n, which buys enough error budget to run 2 of the 8
h-channel k-tiles in fp8 DoubleRow as well (sim rel-err 1.80e-2 vs the
2e-2 budget; x-only fp8 without the split measures 1.66e-2).

M1 thus runs 10 of 16 k-tiles (8 x + 2 h) in fp8 e4m3 DoubleRow (2x PE
throughput per tile), the rest fp16.  All M1 operands carry a uniform
2^10 product scale (x*16 folded into the host transpose, h*16 folded
into the decay matrix, W1*64 folded on host) so fp8 values sit in
e4m3's normal range; the descale 2^-10 rides the |.| activation.  The
EMA output is written both as f16 (A path + f16 M1 tiles) and fp8 (its
first two d-tiles, for the DR pair).

W1 is fully resident in SBUF (11-12 MB) -- loaded once at startup
instead of re-streamed per token block, which removes the DMA stalls in
front of each rep's fp8 matmuls.  M2' (logits) is fused into the M1 loop
per (block, h-tile): as soon as a hid tile is ready it is contracted
into a per-block PSUM logits accumulator, so hid needs only a few
rotating 128x512 tiles instead of a full-window 8 MB buffer.
"""

import sys

if "/opt/trn_rl_repo" not in sys.path:
    sys.path.insert(0, "/opt/trn_rl_repo")

import numpy as np
import ml_dtypes

# Problem shapes (hardcoded per harness contract)
B, T, D, E, H = 8, 2048, 1024, 64, 4096
TP = T  # padded token count (2047 real + 1 pad)
BLK = 512  # token block (psum free-dim)
LB = 128  # lookback tokens
SC = (LB + BLK) // 128  # 5 s-chunks per block
NBLK = TP // BLK  # 4 blocks
NDT = D // 128  # 8 d-tiles
NHT = H // 128  # 32 hid-tiles
NKT = (2 * D) // 128  # 16 k-tiles for M1
BPW = 2  # 512-blocks per window
NW = NBLK // BPW  # 2 windows
WTOK = BPW * BLK  # tokens per window

NF8 = 8  # x-channel d-tiles (all 8) quantized to fp8 e4m3
NF8H = 2  # h-channel d-tiles in fp8 (error budget caps this at 2; even)
NP8 = (NF8 + NF8H) // 2  # DoubleRow pairs
NW16 = NKT - NF8 - NF8H  # f16 k-tiles (h-tiles NF8H..7)

XS = 16.0  # activation scale 2^4 (both x and h)
WS = 64.0  # W1 scale 2^6
DESCALE = 1.0 / (XS * WS)


def _build_program():
    import concourse.bacc as bacc
    import concourse.mybir as mybir
    import concourse.tile as tile
    from concourse._compat import axon_active

    f32 = mybir.dt.float32
    f16 = mybir.dt.float16
    f8 = mybir.dt.float8e4
    AF = mybir.ActivationFunctionType
    AX = mybir.AxisListType
    DR = mybir.MatmulPerfMode.DoubleRow

    nc = bacc.Bacc("TRN2", target_bir_lowering=False, debug=not axon_active())

    # --- DRAM I/O ---
    x_pad = nc.dram_tensor("x_pad", [LB + TP, D], f16, kind="ExternalInput")
    mdec = nc.dram_tensor("mdec", [SC * 128, BLK], f16, kind="ExternalInput")
    xt8 = nc.dram_tensor("xt8", [NF8 * 128, TP], f8, kind="ExternalInput")
    xt16 = nc.dram_tensor("xt16", [NDT * 128, TP], f16, kind="ExternalInput")
    w1t8 = nc.dram_tensor("w1t8", [NHT, 128, NP8 * 256], f8, kind="ExternalInput")
    w1t16 = nc.dram_tensor(
        "w1t16", [NHT, 128, NW16 * 128], f16, kind="ExternalInput"
    )
    b1t = nc.dram_tensor("b1t", [128, NHT], f32, kind="ExternalInput")
    wct = nc.dram_tensor("wct", [128, NHT * E], f16, kind="ExternalInput")
    att = nc.dram_tensor("att", [128, NKT * E], f16, kind="ExternalInput")
    bcr = nc.dram_tensor("bcr", [128, E], f16, kind="ExternalInput")
    onesb = nc.dram_tensor("onesb", [128, 128], f16, kind="ExternalInput")
    out = nc.dram_tensor("out", [TP, E], f32, kind="ExternalOutput")

    with tile.TileContext(nc) as tc:
        with (
            tc.tile_pool(name="const", bufs=1) as cpool,
            tc.tile_pool(name="xin", bufs=2) as xpool,
            tc.tile_pool(name="xtin", bufs=1) as xtpool,
            tc.tile_pool(name="acts", bufs=1) as apool,
            tc.tile_pool(name="hid", bufs=4) as hpool,
            tc.tile_pool(name="sm", bufs=2) as smpool,
            tc.tile_pool(name="ema_ps", bufs=2, space="PSUM") as ema_ps,
            tc.tile_pool(name="m1_ps", bufs=3, space="PSUM") as m1_ps,
            tc.tile_pool(name="m3_ps", bufs=2, space="PSUM") as m3_ps,
        ):
            m_sb = cpool.tile([128, SC * BLK], f16, tag="mdec")
            b1_sb = cpool.tile([128, NHT], f32, tag="b1")
            wc_sb = cpool.tile([128, NHT * E], f16, tag="wc")
            at_sb = cpool.tile([128, NKT * E], f16, tag="at")
            bcr_sb = cpool.tile([128, E], f16, tag="bcr")
            ones_sb = cpool.tile([128, 128], f16, tag="ones")
            w18_sb = cpool.tile([128, NHT * NP8 * 256], f8, tag="w18")
            w116_sb = cpool.tile([128, NHT * NW16 * 128], f16, tag="w116")

            NSR = LB // 128 + BPW * (BLK // 128)  # x rows (128-chunks) per window
            for w in range(NW):
                t0 = w * WTOK
                # --- stream inputs for this window.  On window 0 the DMA ring
                # is the critical path: interleave (x chunk, mdec chunk) pairs
                # in EMA-consumption order so the first EMA matmul starts
                # after ~2 transfers instead of the whole input stream. ---
                x_sb = xpool.tile([128, NSR * D], f16, tag="x")
                for sc in range(NSR):
                    # two half-chunk transfers: EMA matmuls depend on subtile
                    # ranges, so the first d-tiles unblock after half the data
                    for hh in range(2):
                        nc.sync.dma_start(
                            x_sb[:, sc * D + hh * (D // 2) : sc * D + (hh + 1) * (D // 2)],
                            x_pad[
                                t0 + sc * 128 : t0 + (sc + 1) * 128,
                                hh * (D // 2) : (hh + 1) * (D // 2),
                            ],
                        )
                    if w == 0 and sc < SC:
                        nc.sync.dma_start(
                            m_sb[:, sc * BLK : (sc + 1) * BLK],
                            mdec[sc * 128 : (sc + 1) * 128, :],
                        )
                # DMA order matches tensor consumption order (the window-0
                # chain is serial on one ring): consts + xt16 feed the bc/A
                # matmuls that open each block, xt8 feeds the DR matmuls a
                # couple of microseconds later, W1 streams behind per-ht.
                if w == 0:
                    nc.sync.dma_start(b1_sb[:], b1t[:])
                    nc.sync.dma_start(wc_sb[:], wct[:])
                    nc.sync.dma_start(at_sb[:], att[:])
                    nc.sync.dma_start(bcr_sb[:], bcr[:])
                    nc.sync.dma_start(ones_sb[:], onesb[:])
                # per-block halves, block 0 first: block 0's A-path/DR matmuls
                # unblock before block 1's data lands
                xt16_sb = xtpool.tile([128, NDT * WTOK], f16, tag="xt16")
                for bh in range(BPW):
                    for dt in range(NDT):
                        nc.sync.dma_start(
                            xt16_sb[:, dt * WTOK + bh * BLK : dt * WTOK + (bh + 1) * BLK],
                            xt16[
                                dt * 128 : (dt + 1) * 128,
                                t0 + bh * BLK : t0 + (bh + 1) * BLK,
                            ],
                        )
                if w == 0:
                    # First batch's f16 weights land before the xt8 bulk so
                    # M1 isn't gated on the whole input stream.
                    for ht in range(2):
                        nc.sync.dma_start(
                            w116_sb[:, ht * NW16 * 128 : (ht + 1) * NW16 * 128],
                            w1t16[ht, :, :],
                        )
                xt8_sb = xtpool.tile([128, NF8 * WTOK], f8, tag="xt8")
                for bh in range(BPW):
                    for dt in range(NF8):
                        nc.sync.dma_start(
                            xt8_sb[:, dt * WTOK + bh * BLK : dt * WTOK + (bh + 1) * BLK],
                            xt8[
                                dt * 128 : (dt + 1) * 128,
                                t0 + bh * BLK : t0 + (bh + 1) * BLK,
                            ],
                        )
                if w == 0:
                    # W1 resident load: per-ht chunks in consumption order so
                    # M1 starts as soon as each slice lands.
                    for ht in range(2):
                        nc.sync.dma_start(
                            w18_sb[:, ht * NP8 * 256 : (ht + 1) * NP8 * 256],
                            w1t8[ht, :, :],
                        )
                    for ht in range(2, NHT):
                        nc.sync.dma_start(
                            w116_sb[:, ht * NW16 * 128 : (ht + 1) * NW16 * 128],
                            w1t16[ht, :, :],
                        )
                        nc.sync.dma_start(
                            w18_sb[:, ht * NP8 * 256 : (ht + 1) * NP8 * 256],
                            w1t8[ht, :, :],
                        )

                # --- EMA -> hT (channel-major, = 16*h in fp16).  beta^96~4e-5
                # so each s-chunk only streams the ~96-token range it can
                # reach; sc3 goes first: its start=True marks the whole 2KB
                # PSUM bank pending-zero, so every other chunk accumulates
                # over a zeroed tile with only its reachable band. ---
                EMA_PLAN = [
                    (0, 0, 96, True),
                    (1, 0, 216, False),
                    (2, 128, 352, False),
                    (3, 256, 480, False),
                    (4, 384, BLK, False),
                ]
                ht_sb = apool.tile([128, NDT * WTOK], f16, tag="ht")
                ht8_sb = apool.tile([128, NF8H * WTOK], f8, tag="ht8")
                for blk in range(BPW):
                    for dt in range(NDT):
                        ps = ema_ps.tile([128, BLK], f32, tag="ema")
                        for i, (sc, lo, hi, st) in enumerate(EMA_PLAN):
                            off = (blk * (BLK // 128) + sc) * D
                            nc.tensor.matmul(
                                ps[:, lo:hi],
                                x_sb[:, off + dt * 128 : off + (dt + 1) * 128],
                                m_sb[:, sc * BLK + lo : sc * BLK + hi],
                                start=st,
                                stop=(i == len(EMA_PLAN) - 1),
                            )
                        nc.vector.tensor_copy(
                            ht_sb[:, dt * WTOK + blk * BLK : dt * WTOK + (blk + 1) * BLK],
                            ps[:],
                        )
                        if dt < NF8H:
                            nc.vector.tensor_copy(
                                ht8_sb[
                                    :,
                                    dt * WTOK + blk * BLK : dt * WTOK + (blk + 1) * BLK,
                                ],
                                ps[:],
                            )

                # --- M1 + fused M2' per (blk, ht): as soon as hid[ht] is
                # ready, contract it into the block's logits accumulator
                # ps3 [128tok x 4tt*E].  ps3's first matmul zeroes the whole
                # bank (start=True), last (ht=NHT-1, tt=3) stops the group. ---
                # --- M1 in batches of KB h-tiles: each rep's 8 f16 matmuls
                # run for the whole batch, then the batch's fp8 DR matmuls.
                # The tensor engine pays ~190ns to enter fp8 mode after f16,
                # so batching cuts that penalty from once per rep to once per
                # batch.  M2' for the previous batch is emitted after the DR
                # run (its relu has had a full batch of matmuls to finish). ---
                KB = 2
                for blk in range(BPW):
                    bt0 = t0 + blk * BLK
                    ps3 = m3_ps.tile([128, (BLK // 128) * E], f32, tag="m3")
                    # bc folded in as a rank-1 matmul: ones^T @ (bc/128).
                    for tt in range(BLK // 128):
                        nc.tensor.matmul(
                            ps3[:, tt * E : (tt + 1) * E],
                            ones_sb[:],
                            bcr_sb[:],
                            start=(tt == 0),
                            stop=False,
                        )
                    # A path: (1/2) Wc W1 [x;h], 16 k-tile matmuls per token
                    # tile, f16 -- the exact half of the abs split.
                    for ct in range(NKT):
                        src = xt16_sb if ct < NDT else ht_sb
                        coff = (ct if ct < NDT else ct - NDT) * WTOK + blk * BLK
                        for tt in range(BLK // 128):
                            nc.tensor.matmul(
                                ps3[:, tt * E : (tt + 1) * E],
                                src[:, coff + tt * 128 : coff + (tt + 1) * 128],
                                at_sb[:, ct * E : (ct + 1) * E],
                                start=False,
                                stop=False,
                            )

                    def emit_m2(hid_ap, hti):
                        for tt in range(BLK // 128):
                            nc.tensor.matmul(
                                ps3[:, tt * E : (tt + 1) * E],
                                hid_ap[:, tt * 128 : (tt + 1) * 128],
                                wc_sb[:, hti * E : (hti + 1) * E],
                                start=False,
                                stop=(hti == NHT - 1 and tt == BLK // 128 - 1),
                            )

                    prev = []
                    for h0 in range(0, NHT, KB):
                        batch = range(h0, min(h0 + KB, NHT))
                        ps1s = {}
                        for ht in batch:
                            ps1s[ht] = m1_ps.tile(
                                [128, BLK], f32, tag="m1", name="ps1"
                            )
                            for c in range(NW16):
                                doff = (NF8H + c) * WTOK
                                nc.tensor.matmul(
                                    ps1s[ht][:],
                                    w116_sb[
                                        :,
                                        ht * NW16 * 128
                                        + c * 128 : ht * NW16 * 128
                                        + (c + 1) * 128,
                                    ],
                                    ht_sb[
                                        :, doff + blk * BLK : doff + (blk + 1) * BLK
                                    ],
                                    start=(c == 0),
                                    stop=False,
                                )
                        hids = []
                        for ht in batch:
                            for j in range(NP8):
                                mv = xt8_sb if j < NF8 // 2 else ht8_sb
                                moff = (2 * j) if j < NF8 // 2 else 0
                                nc.tensor.matmul(
                                    ps1s[ht][:],
                                    w18_sb[
                                        :,
                                        ht * NP8 * 256 + j * 256 : ht * NP8 * 256
                                        + (j + 1) * 256,
                                    ].rearrange("p (two m) -> p two m", m=128),
                                    mv[:, moff * WTOK : (moff + 2) * WTOK]
                                    .rearrange("p (dt w) -> p dt w", w=WTOK)[
                                        :, :, blk * BLK : (blk + 1) * BLK
                                    ],
                                    start=False,
                                    stop=(j == NP8 - 1),
                                    perf_mode=DR,
                                )
                            hid_sb = hpool.tile([128, BLK], f16, tag="hid")
                            nc.scalar.activation(
                                hid_sb[:],
                                ps1s[ht][:],
                                AF.Abs,
                                bias=b1_sb[:, ht : ht + 1],
                                scale=DESCALE,
                            )
                            hids.append((hid_sb, ht))
                        for hp in prev:
                            emit_m2(*hp)
                        prev = hids
                    for hp in prev:
                        emit_m2(*hp)

                    # --- softmax on ps3.  Logits are O(1) here so exp runs
                    # without the max-subtraction; exp+row-sum fused in one
                    # scalar op (accum_out), recip on Vector, normalize
                    # (Copy with scale=rcp) on Scalar. ---
                    ot = smpool.tile([128, (BLK // 128) * E], f32, tag="ot")
                    for tt in range(BLK // 128):
                        ex = smpool.tile([128, E], f32, tag="ex")
                        ssum = smpool.tile([128, 1], f32, tag="ssum")
                        nc.scalar.activation(
                            ex[:], ps3[:, tt * E : (tt + 1) * E], AF.Exp,
                            accum_out=ssum[:],
                        )
                        rcp = smpool.tile([128, 1], f32, tag="rcp")
                        nc.vector.reciprocal(rcp[:], ssum[:])
                        nc.scalar.activation(
                            ot[:, tt * E : (tt + 1) * E], ex[:], AF.Copy, scale=rcp[:]
                        )
                    # single DMA per block: [4 tok-tiles, 128, E]
                    nc.sync.dma_start(
                        out[bt0 : bt0 + BLK, :].rearrange("(tt p) e -> p tt e", p=128),
                        ot[:].rearrange("p (tt e) -> p tt e", e=E),
                    )

    nc.compile()
    return nc


_prepared = {}


def _prepare_host_inputs(seq, beta_raw, W1, b1, W2, b2, Wr, br):
    f8np = ml_dtypes.float8_e4m3
    seq = np.asarray(seq, np.float32)
    beta = 1.0 / (1.0 + np.exp(-np.asarray(beta_raw, np.float64)))
    assert beta.max() - beta.min() < 1e-6, "kernel assumes channel-constant beta"
    b = float(beta[0])
    assert b ** LB < 1e-4, "lookback too short for this beta"

    x = seq[:, : T - 1, :]  # [B, 2047, D]

    # decay matrix: mdec[s, t] = b^((t+LB)-s) for (t+LB)>=s else 0; carries the
    # 2^4 h-scale so the EMA output lands pre-scaled for M1
    s_idx = np.arange(LB + BLK)[:, None]
    t_idx = np.arange(BLK)[None, :]
    expo = (t_idx + LB) - s_idx
    mdec = (XS * np.where(expo >= 0, b ** np.maximum(expo, 0), 0.0)).astype(np.float16)

    W1 = np.asarray(W1, np.float32)
    b1 = np.asarray(b1, np.float32)
    W1s = W1 * WS
    W2 = np.asarray(W2, np.float32)
    Wr = np.asarray(Wr, np.float32)
    # fold router into predictor layer 2: logits = hid @ (Wr@W2)^T + (br + Wr@b2)
    Wc = (Wr @ W2).astype(np.float32)  # [E, H]
    bc_eff = np.asarray(br, np.float32) + Wr @ np.asarray(b2, np.float32)

    # fp8 k-tiles: x tiles 0..7 plus h tiles 0..NF8H-1, DoubleRow pair layout
    # w1t8[ht, k, j*256 + i*128 + m] = W1k[ht*128+m, (2j+i)*128+k]
    W1k = np.concatenate([W1s[:, : NF8 * 128], W1s[:, D : D + NF8H * 128]], axis=1)
    w1x = W1k.reshape(NHT, 128, NP8, 2, 128)
    w1t8 = np.ascontiguousarray(
        w1x.transpose(0, 4, 2, 3, 1).reshape(NHT, 128, NP8 * 256)
    ).astype(f8np)
    # f16 k-tiles: h tiles NF8H..7
    w1r = W1s[:, D + NF8H * 128 :].reshape(NHT, 128, NW16, 128)
    w1t16 = np.ascontiguousarray(
        w1r.transpose(0, 3, 2, 1).reshape(NHT, 128, NW16 * 128)
    ).astype(np.float16)
    b1t = np.ascontiguousarray(b1.reshape(NHT, 128).T)
    # abs split: wc' = Wc/2; A' = (Wc @ W1)/2/XS; bc' = bc + (Wc @ b1)/2
    wch = 0.5 * Wc
    wct = np.ascontiguousarray(
        wch.T.reshape(NHT, 128, E).transpose(1, 0, 2).reshape(128, NHT * E)
    ).astype(np.float16)
    A = (0.5 / XS) * (Wc @ W1)  # [E, 2D]
    att = np.ascontiguousarray(
        A.T.reshape(NKT, 128, E).transpose(1, 0, 2).reshape(128, NKT * E)
    ).astype(np.float16)
    bc2 = bc_eff + 0.5 * (Wc @ b1)
    bcr = np.ascontiguousarray(
        np.tile(bc2[None, :] / 128.0, (128, 1))
    ).astype(np.float16)
    onesb = np.ones((128, 128), np.float16)

    shared = dict(
        mdec=mdec, w1t8=w1t8, w1t16=w1t16, b1t=b1t, wct=wct, att=att,
        bcr=bcr, onesb=onesb,
    )
    in_maps = []
    for bi in range(B):
        x_pad = np.zeros((LB + TP, D), np.float16)
        x_pad[LB : LB + T - 1] = x[bi]
        xTs = np.zeros((D, TP), np.float32)
        xTs[:, : T - 1] = x[bi].T * XS
        m = dict(shared)
        m["x_pad"] = x_pad
        m["xt8"] = np.ascontiguousarray(xTs[: NF8 * 128]).astype(f8np)
        m["xt16"] = xTs.astype(np.float16)
        in_maps.append(m)
    return in_maps


def kernel(**inputs):
    from concourse import bass_utils

    if "nc" not in _prepared:
        _prepared["nc"] = _build_program()
    nc = _prepared["nc"]
    in_maps = _prepare_host_inputs(**inputs)
    res = bass_utils.run_bass_kernel_spmd(nc, in_maps, core_ids=list(range(B)))
    outs = np.stack([r["out"] for r in res.results], axis=0)  # [B, TP, E]
    return outs[:, : T - 1, :].astype(np.float32)


# revision 42
# speedup vs baseline: 1.0508x; 1.0508x over previous
"""Trainium2 Bass kernel for nn_AblationRouter (moe_routing).

Computation (per batch row):
  h = EMA(x) with per-channel decay beta (constant 0.9 here)
  hid = relu([x, h] @ W1^T + b1);  route = hid @ W2^T + b2
  gates = softmax(route @ Wr^T + br)

Strategy: data-parallel over B=8 batch rows, one per NeuronCore.
W2 and Wr are both linear with no nonlinearity between them, so they are
folded on the host into Wc = Wr @ W2 [E, H] (and bc = br + Wr @ b2),
which removes the route matmul entirely: logits = hid @ Wc^T + bc.

The EMA recurrence is computed as matmuls against a precomputed decay
matrix over 512-token blocks with a 128-token lookback (beta^128 ~ 1e-6,
negligible vs matmul rounding -> no serial carry chain); the decay
matrix is banded so each 128-row s-chunk only streams the token range it
can reach.  All activations are channel-major ([channel, token]) so the
matmuls chain on the TensorEngine without transposes.

The abs split: relu(z) = z/2 + |z|/2, so
  logits = (1/2)(Wc W1)[x;h] + (1/2) Wc |pre + b1| + (bc + (1/2) Wc b1)
The first term is a rank-64 path (A' = Wc W1 / 2, 16 small matmuls per
token tile, f16 -- exact).  Only the |.| path carries fp8 quantization
noise, and |pre+n|-|pre| = sign(pre)*n has half the noise power of
relu's mask*n, which buys enough error budget to run 2 of the 8
h-channel k-tiles in fp8 DoubleRow as well (sim rel-err 1.80e-2 vs the
2e-2 budget; x-only fp8 without the split measures 1.66e-2).

M1 thus runs 10 of 16 k-tiles (8 x + 2 h) in fp8 e4m3 DoubleRow (2x PE
throughput per tile), the rest fp16.  All M1 operands carry a uniform
2^10 product scale (x*16 folded into the host transpose, h*16 folded
into the decay matrix, W1*64 folded on host) so fp8 values sit in
e4m3's normal range; the descale 2^-10 rides the |.| activation.  The
EMA output is written both as f16 (A path + f16 M1 tiles) and fp8 (its
first two d-tiles, for the DR pair).

W1 is fully resident in SBUF (11-12 MB) -- loaded once at startup
instead of re-streamed per token block, which removes the DMA stalls in
front of each rep's fp8 matmuls.  M2' (logits) is fused into the M1 loop
per (block, h-tile): as soon as a hid tile is ready it is contracted
into a per-block PSUM logits accumulator, so hid needs only a few
rotating 128x512 tiles instead of a full-window 8 MB buffer.
"""

import sys

if "/opt/trn_rl_repo" not in sys.path:
    sys.path.insert(0, "/opt/trn_rl_repo")

import numpy as np
import ml_dtypes

# Problem shapes (hardcoded per harness contract)
B, T, D, E, H = 8, 2048, 1024, 64, 4096
TP = T  # padded token count (2047 real + 1 pad)
BLK = 512  # token block (psum free-dim)
LB = 128  # lookback tokens
SC = (LB + BLK) // 128  # 5 s-chunks per block
NBLK = TP // BLK  # 4 blocks
NDT = D // 128  # 8 d-tiles
NHT = H // 128  # 32 hid-tiles
NKT = (2 * D) // 128  # 16 k-tiles for M1
BPW = 2  # 512-blocks per window
NW = NBLK // BPW  # 2 windows
WTOK = BPW * BLK  # tokens per window

NF8 = 8  # x-channel d-tiles (all 8) quantized to fp8 e4m3
NF8H = 2  # h-channel d-tiles in fp8 (error budget caps this at 2; even)
NP8 = (NF8 + NF8H) // 2  # DoubleRow pairs
NW16 = NKT - NF8 - NF8H  # f16 k-tiles (h-tiles NF8H..7)

XS = 16.0  # activation scale 2^4 (both x and h)
WS = 64.0  # W1 scale 2^6
DESCALE = 1.0 / (XS * WS)


def _build_program():
    import concourse.bacc as bacc
    import concourse.mybir as mybir
    import concourse.tile as tile
    from concourse._compat import axon_active

    f32 = mybir.dt.float32
    f16 = mybir.dt.float16
    f8 = mybir.dt.float8e4
    AF = mybir.ActivationFunctionType
    AX = mybir.AxisListType
    DR = mybir.MatmulPerfMode.DoubleRow

    nc = bacc.Bacc("TRN2", target_bir_lowering=False, debug=not axon_active())

    # --- DRAM I/O ---
    x_pad = nc.dram_tensor("x_pad", [LB + TP, D], f16, kind="ExternalInput")
    mdec = nc.dram_tensor("mdec", [SC * 128, BLK], f16, kind="ExternalInput")
    xt8 = nc.dram_tensor("xt8", [NF8 * 128, TP], f8, kind="ExternalInput")
    xt16 = nc.dram_tensor("xt16", [NDT * 128, TP], f16, kind="ExternalInput")
    w1t8 = nc.dram_tensor("w1t8", [NHT, 128, NP8 * 256], f8, kind="ExternalInput")
    w1t16 = nc.dram_tensor(
        "w1t16", [NHT, 128, NW16 * 128], f16, kind="ExternalInput"
    )
    b1t = nc.dram_tensor("b1t", [128, NHT], f32, kind="ExternalInput")
    wct = nc.dram_tensor("wct", [128, NHT * E], f16, kind="ExternalInput")
    att = nc.dram_tensor("att", [128, NKT * E], f16, kind="ExternalInput")
    bcr = nc.dram_tensor("bcr", [128, E], f16, kind="ExternalInput")
    onesb = nc.dram_tensor("onesb", [128, 128], f16, kind="ExternalInput")
    out = nc.dram_tensor("out", [TP, E], f32, kind="ExternalOutput")

    with tile.TileContext(nc) as tc:
        with (
            tc.tile_pool(name="const", bufs=1) as cpool,
            tc.tile_pool(name="xin", bufs=2) as xpool,
            tc.tile_pool(name="xtin", bufs=1) as xtpool,
            tc.tile_pool(name="acts", bufs=1) as apool,
            tc.tile_pool(name="hid", bufs=4) as hpool,
            tc.tile_pool(name="sm", bufs=2) as smpool,
            tc.tile_pool(name="ema_ps", bufs=2, space="PSUM") as ema_ps,
            tc.tile_pool(name="m1_ps", bufs=3, space="PSUM") as m1_ps,
            tc.tile_pool(name="m3_ps", bufs=2, space="PSUM") as m3_ps,
        ):
            m_sb = cpool.tile([128, SC * BLK], f16, tag="mdec")
            b1_sb = cpool.tile([128, NHT], f32, tag="b1")
            wc_sb = cpool.tile([128, NHT * E], f16, tag="wc")
            at_sb = cpool.tile([128, NKT * E], f16, tag="at")
            bcr_sb = cpool.tile([128, E], f16, tag="bcr")
            ones_sb = cpool.tile([128, 128], f16, tag="ones")
            w18_sb = cpool.tile([128, NHT * NP8 * 256], f8, tag="w18")
            w116_sb = cpool.tile([128, NHT * NW16 * 128], f16, tag="w116")

            NSR = LB // 128 + BPW * (BLK // 128)  # x rows (128-chunks) per window
            for w in range(NW):
                t0 = w * WTOK
                # --- stream inputs for this window.  On window 0 the DMA ring
                # is the critical path: interleave (x chunk, mdec chunk) pairs
                # in EMA-consumption order so the first EMA matmul starts
                # after ~2 transfers instead of the whole input stream. ---
                # DMA order matches tensor consumption order (the window-0
                # chain is serial on one ring).  Block 1's x chunks and the
                # const/A-path feeds come after the first M1 batches' weights
                # so block-0 compute starts as early as possible; EMA for
                # block 1 runs after block 0's compute, hiding the tail of
                # the input stream under M1.
                x_sb = xpool.tile([128, NSR * D], f16, tag="x")
                nsr_first = SC if w == 0 else NSR
                for sc in range(nsr_first):
                    nc.sync.dma_start(
                        x_sb[:, sc * D : (sc + 1) * D],
                        x_pad[t0 + sc * 128 : t0 + (sc + 1) * 128, :],
                    )
                    if w == 0 and sc < SC:
                        nc.sync.dma_start(
                            m_sb[:, sc * BLK : (sc + 1) * BLK],
                            mdec[sc * 128 : (sc + 1) * 128, :],
                        )
                if w == 0:
                    nc.sync.dma_start(b1_sb[:], b1t[:])
                    for ht in range(2):
                        nc.sync.dma_start(
                            w116_sb[:, ht * NW16 * 128 : (ht + 1) * NW16 * 128],
                            w1t16[ht, :, :],
                        )
                    for ht in range(2):
                        nc.sync.dma_start(
                            w18_sb[:, ht * NP8 * 256 : (ht + 1) * NP8 * 256],
                            w1t8[ht, :, :],
                        )
                xt8_sb = xtpool.tile([128, NF8 * WTOK], f8, tag="xt8")
                for dt in range(NF8):
                    nc.sync.dma_start(
                        xt8_sb[:, dt * WTOK : (dt + 1) * WTOK],
                        xt8[dt * 128 : (dt + 1) * 128, t0 : t0 + WTOK],
                    )
                if w == 0:
                    for ht in range(2, 7):
                        nc.sync.dma_start(
                            w116_sb[:, ht * NW16 * 128 : (ht + 1) * NW16 * 128],
                            w1t16[ht, :, :],
                        )
                        nc.sync.dma_start(
                            w18_sb[:, ht * NP8 * 256 : (ht + 1) * NP8 * 256],
                            w1t8[ht, :, :],
                        )
                    nc.sync.dma_start(wc_sb[:], wct[:])
                    nc.sync.dma_start(at_sb[:], att[:])
                    nc.sync.dma_start(bcr_sb[:], bcr[:])
                    nc.sync.dma_start(ones_sb[:], onesb[:])
                xt16_sb = xtpool.tile([128, NDT * WTOK], f16, tag="xt16")
                for dt in range(NDT):
                    nc.sync.dma_start(
                        xt16_sb[:, dt * WTOK : (dt + 1) * WTOK],
                        xt16[dt * 128 : (dt + 1) * 128, t0 : t0 + WTOK],
                    )
                if w == 0:
                    for ht in range(7, NHT):
                        nc.sync.dma_start(
                            w116_sb[:, ht * NW16 * 128 : (ht + 1) * NW16 * 128],
                            w1t16[ht, :, :],
                        )
                        nc.sync.dma_start(
                            w18_sb[:, ht * NP8 * 256 : (ht + 1) * NP8 * 256],
                            w1t8[ht, :, :],
                        )
                    for sc in range(SC, NSR):
                        nc.sync.dma_start(
                            x_sb[:, sc * D : (sc + 1) * D],
                            x_pad[t0 + sc * 128 : t0 + (sc + 1) * 128, :],
                        )

                # --- EMA -> hT (channel-major, = 16*h in fp16).  beta^96~4e-5
                # so each s-chunk only streams the ~96-token range it can
                # reach; sc3 goes first: its start=True marks the whole 2KB
                # PSUM bank pending-zero, so every other chunk accumulates
                # over a zeroed tile with only its reachable band. ---
                EMA_PLAN = [
                    (0, 0, 96, True),
                    (1, 0, 216, False),
                    (2, 128, 352, False),
                    (3, 256, 480, False),
                    (4, 384, BLK, False),
                ]
                ht_sb = apool.tile([128, NDT * WTOK], f16, tag="ht")
                ht8_sb = apool.tile([128, NF8H * WTOK], f8, tag="ht8")

                def emit_ema(blk):
                    for dt in range(NDT):
                        ps = ema_ps.tile([128, BLK], f32, tag="ema", name="ps")
                        for i, (sc, lo, hi, st) in enumerate(EMA_PLAN):
                            off = (blk * (BLK // 128) + sc) * D
                            nc.tensor.matmul(
                                ps[:, lo:hi],
                                x_sb[:, off + dt * 128 : off + (dt + 1) * 128],
                                m_sb[:, sc * BLK + lo : sc * BLK + hi],
                                start=st,
                                stop=(i == len(EMA_PLAN) - 1),
                            )
                        nc.vector.tensor_copy(
                            ht_sb[:, dt * WTOK + blk * BLK : dt * WTOK + (blk + 1) * BLK],
                            ps[:],
                        )
                        if dt < NF8H:
                            nc.vector.tensor_copy(
                                ht8_sb[
                                    :,
                                    dt * WTOK + blk * BLK : dt * WTOK + (blk + 1) * BLK,
                                ],
                                ps[:],
                            )

                # --- M1 + fused M2' per (blk, ht): as soon as hid[ht] is
                # ready, contract it into the block's logits accumulator
                # ps3 [128tok x 4tt*E].  ps3's first matmul zeroes the whole
                # bank (start=True), last (ht=NHT-1, tt=3) stops the group. ---
                # --- M1 in batches of KB h-tiles: each rep's 8 f16 matmuls
                # run for the whole batch, then the batch's fp8 DR matmuls.
                # The tensor engine pays ~190ns to enter fp8 mode after f16,
                # so batching cuts that penalty from once per rep to once per
                # batch.  M2' for the previous batch is emitted after the DR
                # run (its relu has had a full batch of matmuls to finish). ---
                KB = 2
                for blk in range(BPW):
                    bt0 = t0 + blk * BLK
                    emit_ema(blk)
                    ps3 = m3_ps.tile([128, (BLK // 128) * E], f32, tag="m3")

                    def emit_bc_a():
                        # bc folded in as a rank-1 matmul: ones^T @ (bc/128);
                        # its start=True opens the ps3 group.  Emitted after
                        # the first M1 batch so window-0 M1 isn't queued
                        # behind the xt16/at transfers this path reads.
                        for tt in range(BLK // 128):
                            nc.tensor.matmul(
                                ps3[:, tt * E : (tt + 1) * E],
                                ones_sb[:],
                                bcr_sb[:],
                                start=(tt == 0),
                                stop=False,
                            )
                        # A path: (1/2) Wc W1 [x;h], 16 k-tile matmuls per
                        # token tile, f16 -- the exact half of the abs split.
                        for ct in range(NKT):
                            src = xt16_sb if ct < NDT else ht_sb
                            coff = (ct if ct < NDT else ct - NDT) * WTOK + blk * BLK
                            for tt in range(BLK // 128):
                                nc.tensor.matmul(
                                    ps3[:, tt * E : (tt + 1) * E],
                                    src[:, coff + tt * 128 : coff + (tt + 1) * 128],
                                    at_sb[:, ct * E : (ct + 1) * E],
                                    start=False,
                                    stop=False,
                                )

                    def emit_m2(hid_ap, hti):
                        for tt in range(BLK // 128):
                            nc.tensor.matmul(
                                ps3[:, tt * E : (tt + 1) * E],
                                hid_ap[:, tt * 128 : (tt + 1) * 128],
                                wc_sb[:, hti * E : (hti + 1) * E],
                                start=False,
                                stop=(hti == NHT - 1 and tt == BLK // 128 - 1),
                            )

                    prev = []
                    for h0 in range(0, NHT, KB):
                        batch = range(h0, min(h0 + KB, NHT))
                        ps1s = {}
                        for ht in batch:
                            ps1s[ht] = m1_ps.tile(
                                [128, BLK], f32, tag="m1", name="ps1"
                            )
                            for c in range(NW16):
                                doff = (NF8H + c) * WTOK
                                nc.tensor.matmul(
                                    ps1s[ht][:],
                                    w116_sb[
                                        :,
                                        ht * NW16 * 128
                                        + c * 128 : ht * NW16 * 128
                                        + (c + 1) * 128,
                                    ],
                                    ht_sb[
                                        :, doff + blk * BLK : doff + (blk + 1) * BLK
                                    ],
                                    start=(c == 0),
                                    stop=False,
                                )
                        hids = []
                        for ht in batch:
                            for j in range(NP8):
                                mv = xt8_sb if j < NF8 // 2 else ht8_sb
                                moff = (2 * j) if j < NF8 // 2 else 0
                                nc.tensor.matmul(
                                    ps1s[ht][:],
                                    w18_sb[
                                        :,
                                        ht * NP8 * 256 + j * 256 : ht * NP8 * 256
                                        + (j + 1) * 256,
                                    ].rearrange("p (two m) -> p two m", m=128),
                                    mv[:, moff * WTOK : (moff + 2) * WTOK]
                                    .rearrange("p (dt w) -> p dt w", w=WTOK)[
                                        :, :, blk * BLK : (blk + 1) * BLK
                                    ],
                                    start=False,
                                    stop=(j == NP8 - 1),
                                    perf_mode=DR,
                                )
                            hid_sb = hpool.tile([128, BLK], f16, tag="hid")
                            nc.scalar.activation(
                                hid_sb[:],
                                ps1s[ht][:],
                                AF.Abs,
                                bias=b1_sb[:, ht : ht + 1],
                                scale=DESCALE,
                            )
                            hids.append((hid_sb, ht))
                        if h0 == KB:
                            emit_bc_a()
                        for hp in prev:
                            emit_m2(*hp)
                        prev = hids
                    for hp in prev:
                        emit_m2(*hp)

                    # --- softmax on ps3.  Logits are O(1) here so exp runs
                    # without the max-subtraction; exp+row-sum fused in one
                    # scalar op (accum_out), recip on Vector, normalize
                    # (Copy with scale=rcp) on Scalar. ---
                    ot = smpool.tile([128, (BLK // 128) * E], f32, tag="ot")
                    for tt in range(BLK // 128):
                        ex = smpool.tile([128, E], f32, tag="ex")
                        ssum = smpool.tile([128, 1], f32, tag="ssum")
                        nc.scalar.activation(
                            ex[:], ps3[:, tt * E : (tt + 1) * E], AF.Exp,
                            accum_out=ssum[:],
                        )
                        rcp = smpool.tile([128, 1], f32, tag="rcp")
                        nc.vector.reciprocal(rcp[:], ssum[:])
                        nc.scalar.activation(
                            ot[:, tt * E : (tt + 1) * E], ex[:], AF.Copy, scale=rcp[:]
                        )
                    # single DMA per block: [4 tok-tiles, 128, E]
                    nc.sync.dma_start(
                        out[bt0 : bt0 + BLK, :].rearrange("(tt p) e -> p tt e", p=128),
                        ot[:].rearrange("p (tt e) -> p tt e", e=E),
                    )

    nc.compile()
    return nc


_prepared = {}


def _prepare_host_inputs(seq, beta_raw, W1, b1, W2, b2, Wr, br):
    f8np = ml_dtypes.float8_e4m3
    seq = np.asarray(seq, np.float32)
    beta = 1.0 / (1.0 + np.exp(-np.asarray(beta_raw, np.float64)))
    assert beta.max() - beta.min() < 1e-6, "kernel assumes channel-constant beta"
    b = float(beta[0])
    assert b ** LB < 1e-4, "lookback too short for this beta"

    x = seq[:, : T - 1, :]  # [B, 2047, D]

    # decay matrix: mdec[s, t] = b^((t+LB)-s) for (t+LB)>=s else 0; carries the
    # 2^4 h-scale so the EMA output lands pre-scaled for M1
    s_idx = np.arange(LB + BLK)[:, None]
    t_idx = np.arange(BLK)[None, :]
    expo = (t_idx + LB) - s_idx
    mdec = (XS * np.where(expo >= 0, b ** np.maximum(expo, 0), 0.0)).astype(np.float16)

    W1 = np.asarray(W1, np.float32)
    b1 = np.asarray(b1, np.float32)
    W1s = W1 * WS
    W2 = np.asarray(W2, np.float32)
    Wr = np.asarray(Wr, np.float32)
    # fold router into predictor layer 2: logits = hid @ (Wr@W2)^T + (br + Wr@b2)
    Wc = (Wr @ W2).astype(np.float32)  # [E, H]
    bc_eff = np.asarray(br, np.float32) + Wr @ np.asarray(b2, np.float32)

    # fp8 k-tiles: x tiles 0..7 plus h tiles 0..NF8H-1, DoubleRow pair layout
    # w1t8[ht, k, j*256 + i*128 + m] = W1k[ht*128+m, (2j+i)*128+k]
    W1k = np.concatenate([W1s[:, : NF8 * 128], W1s[:, D : D + NF8H * 128]], axis=1)
    w1x = W1k.reshape(NHT, 128, NP8, 2, 128)
    w1t8 = np.ascontiguousarray(
        w1x.transpose(0, 4, 2, 3, 1).reshape(NHT, 128, NP8 * 256)
    ).astype(f8np)
    # f16 k-tiles: h tiles NF8H..7
    w1r = W1s[:, D + NF8H * 128 :].reshape(NHT, 128, NW16, 128)
    w1t16 = np.ascontiguousarray(
        w1r.transpose(0, 3, 2, 1).reshape(NHT, 128, NW16 * 128)
    ).astype(np.float16)
    b1t = np.ascontiguousarray(b1.reshape(NHT, 128).T)
    # abs split: wc' = Wc/2; A' = (Wc @ W1)/2/XS; bc' = bc + (Wc @ b1)/2
    wch = 0.5 * Wc
    wct = np.ascontiguousarray(
        wch.T.reshape(NHT, 128, E).transpose(1, 0, 2).reshape(128, NHT * E)
    ).astype(np.float16)
    A = (0.5 / XS) * (Wc @ W1)  # [E, 2D]
    att = np.ascontiguousarray(
        A.T.reshape(NKT, 128, E).transpose(1, 0, 2).reshape(128, NKT * E)
    ).astype(np.float16)
    bc2 = bc_eff + 0.5 * (Wc @ b1)
    bcr = np.ascontiguousarray(
        np.tile(bc2[None, :] / 128.0, (128, 1))
    ).astype(np.float16)
    onesb = np.ones((128, 128), np.float16)

    shared = dict(
        mdec=mdec, w1t8=w1t8, w1t16=w1t16, b1t=b1t, wct=wct, att=att,
        bcr=bcr, onesb=onesb,
    )
    in_maps = []
    for bi in range(B):
        x_pad = np.zeros((LB + TP, D), np.float16)
        x_pad[LB : LB + T - 1] = x[bi]
        xTs = np.zeros((D, TP), np.float32)
        xTs[:, : T - 1] = x[bi].T * XS
        m = dict(shared)
        m["x_pad"] = x_pad
        m["xt8"] = np.ascontiguousarray(xTs[: NF8 * 128]).astype(f8np)
        m["xt16"] = xTs.astype(np.float16)
        in_maps.append(m)
    return in_maps


def kernel(**inputs):
    from concourse import bass_utils

    if "nc" not in _prepared:
        _prepared["nc"] = _build_program()
    nc = _prepared["nc"]
    in_maps = _prepare_host_inputs(**inputs)
    res = bass_utils.run_bass_kernel_spmd(nc, in_maps, core_ids=list(range(B)))
    outs = np.stack([r["out"] for r in res.results], axis=0)  # [B, TP, E]
    return outs[:, : T - 1, :].astype(np.float32)
